# revision 24
# baseline (speedup 1.0000x reference)
"""Trainium2 Bass kernel for nn_LiquidOperator (preproc MLP -> 4 LTC scans -> 2 MLPs).

Strategy: the LTC cell is strongly contracting (denominator >= 1.067, state
error decays ~0.90x/step), so the 4096-step scan is split into 8 speculative
time segments, one per core. Each core runs its segment plus a W-step warm-up
that starts from h=0; after W=64 steps the warm-up error is ~1e-4 relative,
far under the 2e-2 gate. Warm-up columns before t=0 are masked with a large
negative sigmoid bias, which pins h to exactly 0 until the true t=0.

Each core runs BOTH var-pairs as two interleaved dependency chains (the chains
hide in each other's engine gaps), with each pair's two 56-cell LTCs packed
block-diagonally into one 128x128 stationary matmul weight. The cell keeps
only the sigmoid on the scalar engine; numerator/denominator/divide run on
DVE, which shortens the per-step critical path ~30%.

All constants + the per-core x window are packed into ONE [128, NCOL] input
tensor: per-call dispatch overhead is ~15us per buffer, so 45 buffers -> 1
saves ~0.7ms/call. No collectives: every core ends up with all 4 vars of its
own time shard and runs the output encoders locally. Host does layout only
(slice/pad/transpose/concat).
"""

import numpy as np

import concourse.bass as bass
import concourse.bacc as bacc
import concourse.tile as tile
import concourse.mybir as mybir
from concourse import bass_utils

F32 = mybir.dt.float32
AF = mybir.ActivationFunctionType
OP = mybir.AluOpType

VAR_N, LEVELS, NCELLS, PRED_N = 4, 17, 56, 12
D = VAR_N * LEVELS  # 68
FLAT = VAR_N * D  # 272
T_FULL = 4096
DT = 0.1
N_CORES = 8
NCP = 128  # packed-cell lanes per pair: var-even @ 0..56, var-odd @ 64..120
VOFF = 64
W_DEF = 64  # warm-up steps (error ~0.9^W; 64 -> ~1e-4 rel, gate is 2e-2)
MASKVAL = -30000.0

C1_DIMS = [(LEVELS, LEVELS), (LEVELS, LEVELS), (LEVELS, D), (D, D), (D, D)]
C2_DIMS = [(FLAT, FLAT), (FLAT, FLAT), (FLAT, D), (D, D), (D, D)]
MSPLIT_272 = [(0, 128), (128, 128), (256, 16)]
C2_KSPLITS = {
    1: [(0, 68), (68, 68), (136, 68), (204, 68)],
    2: MSPLIT_272,
    3: MSPLIT_272,
    4: [(0, D)],
    5: [(0, D)],
}


def _chunks(total, step=512):
    off = 0
    while off < total:
        yield off, min(step, total - off)
        off += step


class _Packer:
    """Column-offset bookkeeping for the single packed [128, NCOL] input."""

    def __init__(self):
        self.col = 0
        self.slots = {}  # name -> (part, col0, ncol)

    def add(self, name, parts, ncols):
        self.slots[name] = (parts, self.col, ncols)
        self.col += ncols

    def ap(self, cst, name, p0=0):
        parts, c0, nc_ = self.slots[name]
        return cst[p0 : p0 + parts, c0 : c0 + nc_] if p0 == 0 else None

    def sl(self, cst, name, prow, pn, crow, cn):
        parts, c0, _ = self.slots[name]
        return cst[prow : prow + pn, c0 + crow : c0 + crow + cn]


def _layout(L):
    pk = _Packer()
    pk.add("xw", D, L)
    pk.add("mfl", 128, 1)
    pk.add("pw", D, 5 * D)
    pk.add("pb", D, 5)
    pk.add("wxj", D, 16 * NCP)  # per (var-group, j) [68,128] lhsT blocks
    pk.add("wh", NCP, 2 * NCP)
    pk.add("wx2", VOFF, 2 * NCP)
    pk.add("wout", NCP, 2 * VOFF)
    pk.add("av", NCP, 2)
    pk.add("tau", NCP, 2)
    pk.add("bv", NCP, 2)
    pk.add("bo", VOFF, 2)
    pk.add("Bv", NCP, 2)   # m-state: B = -a*(C-1)
    pk.add("m0", NCP, 2)   # m-state initial value -a*C  (h=0)
    for i, (fi, fo) in enumerate(C1_DIMS, 1):
        pk.add(f"c1w{i}", fi, fo)
    pk.add("c1b", D, 5)
    for i, (fi, fo) in enumerate(C2_DIMS, 1):
        for ki, (ko, kw) in enumerate(C2_KSPLITS[i]):
            pk.add(f"c2w{i}_{ki}", kw, fo)
    for i in (1, 2):
        pk.add(f"c2bs{i}", 128, 3)
    for i in (3, 4, 5):
        pk.add(f"c2b{i}", D, 1)
    return pk


def build(T=T_FULL, n_cores=N_CORES, W=W_DEF, scan_repeat=1, skip_scan=False,
          num_pool=False, pool_mult=False, mstate=True):
    SEG = T // n_cores
    assert SEG % 4 == 0 and W % 4 == 0
    L = W + SEG  # scan steps per chain
    VBl = L // 4  # x-window rows per var
    SEGE = SEG + PRED_N  # encoder width per core
    LT = L + PRED_N
    pk = _layout(L)
    NCOL = pk.col

    nc = bacc.Bacc("TRN2", target_bir_lowering=False, debug=False, num_devices=n_cores)

    d_cst = nc.dram_tensor("cst", [128, NCOL], F32, kind="ExternalInput")
    d_out = nc.dram_tensor("out", [D, SEGE], F32, kind="ExternalOutput")

    with tile.TileContext(nc) as tc:
        with (
            tc.tile_pool(name="const", bufs=1) as cp,
            tc.tile_pool(name="work", bufs=1) as wp,
            tc.tile_pool(name="ps0", bufs=1, space="PSUM") as psc0,
            tc.tile_pool(name="ps1", bufs=1, space="PSUM") as psc1,
            tc.tile_pool(name="ps_big", bufs=4, space="PSUM") as psb,
            tc.tile_pool(name="sm0", bufs=4) as sm0,
            tc.tile_pool(name="sm1", bufs=4) as sm1,
        ):
            psc = [psc0, psc1]
            sm = [sm0, sm1]

            cst = cp.tile([128, NCOL], F32, tag="cst")
            nc.sync.dma_start(cst[:], d_cst.ap())

            def S(name, prow=0, pn=None, crow=0, cn=None):
                parts, c0, ncol = pk.slots[name]
                if pn is None:
                    pn = parts
                if cn is None:
                    cn = ncol
                return cst[prow : prow + pn, c0 + crow : c0 + crow + cn]

            # derived per-cell constants: A = DT*a ; C = 1 + DT/(tau+0.5)
            A_sb = cp.tile([NCP, 2], F32, tag="A")
            nc.vector.tensor_scalar_mul(A_sb[:], S("av"), DT)
            C_sb = cp.tile([NCP, 2], F32, tag="C")
            nc.vector.tensor_scalar_add(C_sb[:], S("tau"), 0.5)
            nc.vector.reciprocal(C_sb[:], C_sb[:])
            nc.vector.tensor_scalar(C_sb[:], C_sb[:], DT, 1.0, op0=OP.mult, op1=OP.add)

            # ---- preproc MLP on the x window (transposed [68, L]) ----
            xt_a = wp.tile([D, L], F32, tag="xt_a")
            xt_b = wp.tile([D, L], F32, tag="xt_b")
            cur, nxt = None, xt_a
            for l in range(5):
                src = S("xw") if l == 0 else cur[:]
                for off, cw in _chunks(L):
                    pt = psb.tile([128, cw], F32, tag="psB")
                    nc.tensor.matmul(
                        pt[:D, :], S("pw", crow=l * D, cn=D), src[:, off : off + cw]
                    )
                    nc.scalar.activation(
                        nxt[:, off : off + cw],
                        pt[:D, :],
                        AF.Relu if l < 4 else AF.Identity,
                        bias=S("pb", crow=l, cn=1),
                    )
                cur, nxt = nxt, (xt_b if nxt is xt_a else xt_a)
            pre_t = cur  # [68, L] = pre(window rows)^T, var blocks of VBl cols

            # ---- UX = xs @ wx + b (+ warm-up mask), per pair, [128, L] ----
            ux = []
            for p in range(2):
                uxp = wp.tile([NCP, L], F32, tag=f"ux{p}")
                nc.vector.memset(uxp[:], 0.0)
                ux3 = uxp[:].rearrange("q (r j) -> q r j", j=4)
                for o in range(2):
                    vg = 2 * p + o
                    rows = slice(o * VOFF, o * VOFF + NCELLS)
                    for j in range(4):
                        s = vg * 4 + j
                        for off, cw in _chunks(VBl):
                            pt = psb.tile([128, cw], F32, tag="psB")
                            nc.tensor.matmul(
                                pt[:],
                                S("wxj", crow=s * NCP, cn=NCP),
                                pre_t[:, vg * VBl + off : vg * VBl + off + cw],
                            )
                            nc.scalar.activation(
                                ux3[rows, off : off + cw, j],
                                pt[rows, :],
                                AF.Identity,
                                bias=S("bv", prow=o * VOFF, pn=NCELLS, crow=p, cn=1),
                            )
                # warm-up mask: mfl = MASKVAL on core 0, 0 elsewhere
                nc.vector.tensor_scalar(
                    uxp[:, :W], uxp[:, :W], 1.0, S("mfl"), op0=OP.mult, op1=OP.add
                )
                ux.append(uxp)

            # ---- LTC scans: FOUR interleaved chains (pair p, half s) ----
            # Each core splits its SEG columns into two halves, each scanned by
            # an independent chain with its own W-step warm-up from h=0 (the
            # second half's warm-up re-reads columns the first half also
            # covers, so the ux window is unchanged). Four chains hide each
            # other's cross-engine latency; the scalar engine's 4 sigmoids per
            # step become the throughput limit instead of chain latency.
            HSEG = SEG // 2
            LP = W + HSEG
            hb = [
                [
                    wp.tile(
                        [NCP, LP + (PRED_N if s == 1 else 0)],
                        F32, tag=f"hbuf{p}_{s}", name=f"hbuf{p}_{s}",
                    )
                    for s in range(2)
                ]
                for p in range(2)
            ]
            h0 = cp.tile([NCP, 1], F32, tag="h0")
            nc.vector.memset(h0[:], 0.0)

            def cell(p, s, hprev, bias_ap, dst, extra_mm=None):
                # m-state form (mstate=True): the recurrence is rewritten in
                # m = h - a*C, which satisfies  m' = m*r + B  with
                # r = 1/(C + DT*f), B = -a*(C-1), and f = sigmoid(wh^T m + ux')
                # where ux' absorbs wh^T(a*C) and the output bias absorbs
                # (a*C)@wout (all folded host-side). This removes the separate
                # numerator op: 3 DVE ops per cell instead of 4 on the hot
                # engine. Exact same fp32 dynamics up to rounding.
                pz = psc[p].tile([NCP, 1], F32, tag=f"psS{p}_{s}")
                if extra_mm is not None:
                    nc.tensor.matmul(
                        pz[:], S("wx2", crow=p * NCP, cn=NCP), extra_mm,
                        start=True, stop=False,
                    )
                nc.tensor.matmul(
                    pz[:], S("wh", crow=p * NCP, cn=NCP), hprev,
                    start=(extra_mm is None), stop=True,
                )
                ft = sm[p].tile([NCP, 1], F32, tag=f"f{p}_{s}")
                nc.scalar.activation(ft[:], pz[:], AF.Sigmoid, bias=bias_ap)
                dent = sm[p].tile([NCP, 1], F32, tag=f"den{p}_{s}")
                nc.vector.tensor_scalar(
                    dent[:], ft[:], DT, C_sb[:, p : p + 1], op0=OP.mult, op1=OP.add
                )
                nc.vector.reciprocal(dent[:], dent[:])
                if mstate:
                    nc.vector.scalar_tensor_tensor(
                        dst, hprev, dent[:], S("Bv", crow=p, cn=1),
                        op0=OP.mult, op1=OP.add,
                    )
                else:
                    numt = sm[p].tile([NCP, 1], F32, tag=f"num{p}_{s}")
                    nc.vector.scalar_tensor_tensor(
                        numt[:], ft[:], A_sb[:, p : p + 1], hprev,
                        op0=OP.mult, op1=OP.add,
                    )
                    eng = nc.gpsimd if pool_mult else nc.vector
                    eng.tensor_tensor(dst, numt[:], dent[:], op=OP.mult)

            if skip_scan:
                for p in range(2):
                    for s in range(2):
                        nc.vector.memset(hb[p][s][:], 0.0)
            for rep in range(0 if skip_scan else scan_repeat):
                for t in range(LP):
                    for s in range(2):
                        for p in range(2):
                            if t == 0:
                                init = S("m0", crow=p, cn=1) if mstate else h0[:]
                                hprev = init if rep == 0 else hb[p][s][:, LP - 1 : LP]
                            else:
                                hprev = hb[p][s][:, t - 1 : t]
                            cell(
                                p, s, hprev,
                                ux[p][:, s * HSEG + t : s * HSEG + t + 1],
                                hb[p][s][:, t : t + 1],
                            )

            # ---- batched output projection of the segment columns ----
            vvt = [
                wp.tile([VOFF, SEGE], F32, tag=f"vvt{p}", name=f"vvt{p}")
                for p in range(2)
            ]
            for p in range(2):
                for s in range(2):
                    for off, cw in _chunks(HSEG):
                        pv = psb.tile([128, cw], F32, tag="psB")
                        nc.tensor.matmul(
                            pv[:VOFF, :],
                            S("wout", crow=p * VOFF, cn=VOFF),
                            hb[p][s][:, W + off : W + off + cw],
                        )
                        nc.scalar.activation(
                            vvt[p][:, s * HSEG + off : s * HSEG + off + cw],
                            pv[:VOFF, :],
                            AF.Identity, bias=S("bo", crow=p, cn=1),
                        )

            # ---- autoregressive prediction (only core 7's result is used) ----
            for i in range(PRED_N):
                for p in range(2):
                    tl = LP + i
                    vprev = vvt[p][:, SEG + i - 1 : SEG + i]
                    cell(
                        p, 1,
                        hb[p][1][:, tl - 1 : tl],
                        S("bv", crow=p, cn=1),
                        hb[p][1][:, tl : tl + 1],
                        extra_mm=vprev,
                    )
                    pv = psc[p].tile([NCP, 1], F32, tag=f"psS{p}_1")
                    nc.tensor.matmul(
                        pv[:VOFF, :], S("wout", crow=p * VOFF, cn=VOFF),
                        hb[p][1][:, tl : tl + 1],
                    )
                    nc.scalar.activation(
                        vvt[p][:, SEG + i : SEG + i + 1], pv[:VOFF, :],
                        AF.Identity, bias=S("bo", crow=p, cn=1),
                    )

            # ---- gather per-var views (var-odd needs a lane move via DMA) ----
            vsh = []
            for p in range(2):
                vsh.append(vvt[p][0:LEVELS, :])
                tv = wp.tile([LEVELS, SEGE], F32, tag=f"vshB{p}")
                nc.sync.dma_start(tv[:], vvt[p][32 : 32 + LEVELS, :])
                vsh.append(tv[:])

            # ---- c1 encoder per var (all 5 layers relu'd: 1-4 inner, 5 outer) ----
            y5 = []
            for v in range(VAR_N):
                src = vsh[v]
                for l in range(1, 6):
                    fo = C1_DIMS[l - 1][1]
                    dst = wp.tile([fo, SEGE], F32, tag=f"c1y{l}_{v}")
                    for off, cw in _chunks(SEGE):
                        pt = psb.tile([128, cw], F32, tag="psB")
                        nc.tensor.matmul(
                            pt[:fo, :], S(f"c1w{l}"), src[:, off : off + cw]
                        )
                        nc.scalar.activation(
                            dst[:, off : off + cw], pt[:fo, :], AF.Relu,
                            bias=S("c1b", pn=fo, crow=l - 1, cn=1),
                        )
                    src = dst[:]
                y5.append(src)  # [68, SEGE]

            # ---- c2 encoder ----
            acts = y5
            for l in range(1, 6):
                fi, fo = C2_DIMS[l - 1]
                msplit = MSPLIT_272 if fo == FLAT else [(0, fo)]
                newacts = []
                for mi, (mo, mw) in enumerate(msplit):
                    dst = wp.tile([mw, SEGE], F32, tag=f"c2z{l}_{mi}")
                    for off, cw in _chunks(SEGE):
                        pt = psb.tile([128, cw], F32, tag="psB")
                        n_k = len(acts)
                        for ki, atile in enumerate(acts):
                            nc.tensor.matmul(
                                pt[:mw, :],
                                S(f"c2w{l}_{ki}", crow=mo, cn=mw),
                                atile[:, off : off + cw],
                                start=(ki == 0),
                                stop=(ki == n_k - 1),
                            )
                        bias = (
                            S(f"c2bs{l}", pn=mw, crow=mi, cn=1)
                            if fo == FLAT
                            else S(f"c2b{l}")
                        )
                        nc.scalar.activation(
                            dst[:, off : off + cw],
                            pt[:mw, :],
                            AF.Relu if l < 5 else AF.Identity,
                            bias=bias,
                        )
                    newacts.append(dst[:])
                acts = newacts

            nc.sync.dma_start(d_out.ap(), acts[0])

    nc.compile()
    return nc, dict(
        T=T, TP=T + PRED_N, SEG=SEG, W=W, L=L, VBl=VBl, SEGE=SEGE,
        n_cores=n_cores, pk=pk, NCOL=NCOL, mstate=mstate,
    )


def make_in_maps(inputs, meta):
    """Host-side layout: pack everything into one [128, NCOL] array per core."""
    T, SEG, W, L, VBl = meta["T"], meta["SEG"], meta["W"], meta["L"], meta["VBl"]
    n_cores, pk, NCOL = meta["n_cores"], meta["pk"], meta["NCOL"]
    g = lambda k: np.ascontiguousarray(np.asarray(inputs[k], dtype=np.float32))

    base = np.zeros((128, NCOL), np.float32)

    def put(name, arr):
        parts, c0, ncol = pk.slots[name]
        assert arr.shape == (parts, ncol), (name, arr.shape, (parts, ncol))
        base[:parts, c0 : c0 + ncol] = arr

    x = g("x")
    put("pw", np.stack([g(f"pw{i}") for i in range(1, 6)]).transpose(1, 0, 2).reshape(D, 5 * D))
    put("pb", np.stack([g(f"pb{i}") for i in range(1, 6)]).T)
    wx_all = g("ltc_wx")  # (4, 17, 56)

    wxj = np.zeros((16, D, NCP), np.float32)
    for vg in range(4):
        for j in range(4):
            wxj[vg * 4 + j, 17 * j : 17 * (j + 1), (vg % 2) * VOFF : (vg % 2) * VOFF + NCELLS] = wx_all[vg]
    put("wxj", wxj.transpose(1, 0, 2).reshape(D, 16 * NCP))

    mstate = meta.get("mstate", True)
    DTc = np.float32(DT)
    WH = np.zeros((NCP, 2, NCP), np.float32)
    WX2P = np.zeros((VOFF, 2, NCP), np.float32)
    WOUTP = np.zeros((NCP, 2, VOFF), np.float32)
    av = np.zeros((NCP, 2), np.float32)
    tau = np.full((NCP, 2), 0.5, np.float32)
    bv = np.zeros((NCP, 2), np.float32)
    bo = np.zeros((VOFF, 2), np.float32)
    Bv = np.zeros((NCP, 2), np.float32)
    m0 = np.zeros((NCP, 2), np.float32)
    for p in range(2):
        for o in range(2):
            v = 2 * p + o
            sl = slice(o * VOFF, o * VOFF + NCELLS)
            WH[sl, p, sl] = g("ltc_wh")[v]
            WX2P[o * 32 : o * 32 + LEVELS, p, sl] = wx_all[v]
            WOUTP[sl, p, o * 32 : o * 32 + LEVELS] = g("ltc_wout")[v]
            av[sl, p] = g("ltc_a")[v]
            tau[sl, p] = g("ltc_tau")[v]
            bvec = g("ltc_b")[v]
            bout = g("ltc_bout")[v]
            if mstate:
                # fold the affine state shift h = m + a*C into the biases
                a_v = g("ltc_a")[v]
                C_v = (1.0 + DTc / (g("ltc_tau")[v] + 0.5)).astype(np.float32)
                aC = (a_v * C_v).astype(np.float32)
                Bv[sl, p] = -a_v * (C_v - 1.0)
                m0[sl, p] = -aC
                bvec = bvec + aC @ g("ltc_wh")[v]
                bout = bout + aC @ g("ltc_wout")[v]
            bv[sl, p] = bvec
            bo[o * 32 : o * 32 + LEVELS, p] = bout
    put("wh", WH.reshape(NCP, 2 * NCP))
    put("wx2", WX2P.reshape(VOFF, 2 * NCP))
    put("wout", WOUTP.reshape(NCP, 2 * VOFF))
    put("av", av)
    put("tau", tau)
    put("bv", bv)
    put("bo", bo)
    put("Bv", Bv)
    put("m0", m0)

    for i in range(1, 6):
        put(f"c1w{i}", g(f"c1w{i}"))
    c1b = np.zeros((D, 5), np.float32)
    for i in range(1, 6):
        b = g(f"c1b{i}")
        c1b[: b.shape[0], i - 1] = b
    put("c1b", c1b)

    for i in range(1, 6):
        w = g(f"c2w{i}")
        for ki, (ko, kw) in enumerate(C2_KSPLITS[i]):
            put(f"c2w{i}_{ki}", w[ko : ko + kw])
    for i in (1, 2):
        b = g(f"c2b{i}")
        bs = np.zeros((128, 3), np.float32)
        for mi, (mo, mw) in enumerate(MSPLIT_272):
            bs[:mw, mi] = b[mo : mo + mw]
        put(f"c2bs{i}", bs)
    for i in (3, 4, 5):
        put(f"c2b{i}", g(f"c2b{i}")[:, None])

    # x reshaped per var: pre row r of var v lives at x row v*(T//4)+r
    TB = T // 4
    maps = []
    for c in range(n_cores):
        m = base.copy()
        t0 = SEG * (c + 1) - L  # window start (may be negative)
        r0 = t0 // 4
        xw = np.zeros((4, VBl, D), np.float32)
        lo = max(0, -r0)
        xw[:, lo:] = x.reshape(4, TB, D)[:, r0 + lo : r0 + VBl]
        parts, c0, ncol = pk.slots["xw"]
        m[:parts, c0 : c0 + ncol] = xw.reshape(4 * VBl, D).T
        parts, c0, ncol = pk.slots["mfl"]
        m[:parts, c0] = MASKVAL if t0 < 0 else 0.0
        maps.append({"cst": m})
    return maps


_CACHE = {}


def _get_built(T=T_FULL):
    if T not in _CACHE:
        _CACHE[T] = build(T)
    return _CACHE[T]


def kernel(**inputs) -> np.ndarray:
    nc, meta = _get_built(T_FULL)
    in_maps = make_in_maps(inputs, meta)
    res = bass_utils.run_bass_kernel_spmd(
        nc, in_maps, core_ids=list(range(meta["n_cores"]))
    )
    SEG = meta["SEG"]
    parts = [res.results[c]["out"][:, :SEG] for c in range(meta["n_cores"] - 1)]
    parts.append(res.results[meta["n_cores"] - 1]["out"])  # includes the 12 pred cols
    full = np.concatenate(parts, axis=1).T  # (T+12, 68)
    return np.ascontiguousarray(full)


# revision 26
# speedup vs baseline: 1.2151x; 1.2151x over previous
"""Trainium2 Bass kernel for nn_LiquidOperator (preproc MLP -> 4 LTC scans -> 2 MLPs).

Strategy: the LTC cell is strongly contracting (denominator >= 1.067, state
error decays ~0.90x/step), so the 4096-step scan is split into 8 speculative
time segments, one per core. Each core runs its segment plus a W-step warm-up
that starts from h=0; after W=64 steps the warm-up error is ~1e-4 relative,
far under the 2e-2 gate. Warm-up columns before t=0 are masked with a large
negative sigmoid bias, which pins h to exactly 0 until the true t=0.

Each core runs BOTH var-pairs as two interleaved dependency chains (the chains
hide in each other's engine gaps), with each pair's two 56-cell LTCs packed
block-diagonally into one 128x128 stationary matmul weight. The cell keeps
only the sigmoid on the scalar engine; numerator/denominator/divide run on
DVE, which shortens the per-step critical path ~30%.

All constants + the per-core x window are packed into ONE [128, NCOL] input
tensor: per-call dispatch overhead is ~15us per buffer, so 45 buffers -> 1
saves ~0.7ms/call. No collectives: every core ends up with all 4 vars of its
own time shard and runs the output encoders locally. Host does layout only
(slice/pad/transpose/concat).
"""

import numpy as np

import concourse.bass as bass
import concourse.bacc as bacc
import concourse.tile as tile
import concourse.mybir as mybir
from concourse import bass_utils

F32 = mybir.dt.float32
AF = mybir.ActivationFunctionType
OP = mybir.AluOpType

VAR_N, LEVELS, NCELLS, PRED_N = 4, 17, 56, 12
D = VAR_N * LEVELS  # 68
FLAT = VAR_N * D  # 272
T_FULL = 4096
DT = 0.1
N_CORES = 8
NCP = 128  # packed-cell lanes per pair: var-even @ 0..56, var-odd @ 64..120
VOFF = 64
W_DEF = 64  # warm-up steps (error ~0.9^W; 64 -> ~1e-4 rel, gate is 2e-2)
MASKVAL = -30000.0

C1_DIMS = [(LEVELS, LEVELS), (LEVELS, LEVELS), (LEVELS, D), (D, D), (D, D)]
C2_DIMS = [(FLAT, FLAT), (FLAT, FLAT), (FLAT, D), (D, D), (D, D)]
MSPLIT_272 = [(0, 128), (128, 128), (256, 16)]
C2_KSPLITS = {
    1: [(0, 68), (68, 68), (136, 68), (204, 68)],
    2: MSPLIT_272,
    3: MSPLIT_272,
    4: [(0, D)],
    5: [(0, D)],
}


def _chunks(total, step=512):
    off = 0
    while off < total:
        yield off, min(step, total - off)
        off += step


class _Packer:
    """Column-offset bookkeeping for the single packed [128, NCOL] input."""

    def __init__(self):
        self.col = 0
        self.slots = {}  # name -> (part, col0, ncol)

    def add(self, name, parts, ncols):
        self.slots[name] = (parts, self.col, ncols)
        self.col += ncols

    def ap(self, cst, name, p0=0):
        parts, c0, nc_ = self.slots[name]
        return cst[p0 : p0 + parts, c0 : c0 + nc_] if p0 == 0 else None

    def sl(self, cst, name, prow, pn, crow, cn):
        parts, c0, _ = self.slots[name]
        return cst[prow : prow + pn, c0 + crow : c0 + crow + cn]


def _layout(L):
    pk = _Packer()
    pk.add("xw", D, L)
    pk.add("mfl", 128, 1)
    pk.add("pw", D, 5 * D)
    pk.add("pb", D, 5)
    pk.add("wxj", D, 16 * NCP)  # per (var-group, j) [68,128] lhsT blocks
    pk.add("wh", NCP, 2 * NCP)
    pk.add("wx2", VOFF, 2 * NCP)
    pk.add("wout", NCP, 2 * VOFF)
    pk.add("av", NCP, 2)
    pk.add("tau", NCP, 2)
    pk.add("bv", NCP, 2)
    pk.add("bo", VOFF, 2)
    pk.add("Bv", NCP, 2)   # m-state: B = -a*(C-1)
    pk.add("m0", NCP, 2)   # m-state initial value -a*C  (h=0)
    for i, (fi, fo) in enumerate(C1_DIMS, 1):
        pk.add(f"c1w{i}", fi, fo)
    pk.add("c1b", D, 5)
    for i, (fi, fo) in enumerate(C2_DIMS, 1):
        for ki, (ko, kw) in enumerate(C2_KSPLITS[i]):
            pk.add(f"c2w{i}_{ki}", kw, fo)
    for i in (1, 2):
        pk.add(f"c2bs{i}", 128, 3)
    for i in (3, 4, 5):
        pk.add(f"c2b{i}", D, 1)
    return pk


def build(T=T_FULL, n_cores=N_CORES, W=W_DEF, scan_repeat=1, skip_scan=False,
          num_pool=False, pool_mult=False, mstate=True, den_act=False):
    SEG = T // n_cores
    assert SEG % 4 == 0 and W % 4 == 0
    L = W + SEG  # scan steps per chain
    VBl = L // 4  # x-window rows per var
    SEGE = SEG + PRED_N  # encoder width per core
    LT = L + PRED_N
    pk = _layout(L)
    NCOL = pk.col

    nc = bacc.Bacc("TRN2", target_bir_lowering=False, debug=False, num_devices=n_cores)

    d_cst = nc.dram_tensor("cst", [128, NCOL], F32, kind="ExternalInput")
    d_out = nc.dram_tensor("out", [D, SEGE], F32, kind="ExternalOutput")

    with tile.TileContext(nc) as tc:
        with (
            tc.tile_pool(name="const", bufs=1) as cp,
            tc.tile_pool(name="work", bufs=1) as wp,
            tc.tile_pool(name="ps0", bufs=1, space="PSUM") as psc0,
            tc.tile_pool(name="ps1", bufs=1, space="PSUM") as psc1,
            tc.tile_pool(name="ps_big", bufs=4, space="PSUM") as psb,
            tc.tile_pool(name="sm0", bufs=4) as sm0,
            tc.tile_pool(name="sm1", bufs=4) as sm1,
        ):
            psc = [psc0, psc1]
            sm = [sm0, sm1]

            cst = cp.tile([128, NCOL], F32, tag="cst")
            nc.sync.dma_start(cst[:], d_cst.ap())

            def S(name, prow=0, pn=None, crow=0, cn=None):
                parts, c0, ncol = pk.slots[name]
                if pn is None:
                    pn = parts
                if cn is None:
                    cn = ncol
                return cst[prow : prow + pn, c0 + crow : c0 + crow + cn]

            # derived per-cell constants: A = DT*a ; C = 1 + DT/(tau+0.5)
            A_sb = cp.tile([NCP, 2], F32, tag="A")
            nc.vector.tensor_scalar_mul(A_sb[:], S("av"), DT)
            C_sb = cp.tile([NCP, 2], F32, tag="C")
            nc.vector.tensor_scalar_add(C_sb[:], S("tau"), 0.5)
            nc.vector.reciprocal(C_sb[:], C_sb[:])
            nc.vector.tensor_scalar(C_sb[:], C_sb[:], DT, 1.0, op0=OP.mult, op1=OP.add)

            # ---- preproc MLP on the x window (transposed [68, L]) ----
            xt_a = wp.tile([D, L], F32, tag="xt_a")
            xt_b = wp.tile([D, L], F32, tag="xt_b")
            cur, nxt = None, xt_a
            for l in range(5):
                src = S("xw") if l == 0 else cur[:]
                for off, cw in _chunks(L):
                    pt = psb.tile([128, cw], F32, tag="psB")
                    nc.tensor.matmul(
                        pt[:D, :], S("pw", crow=l * D, cn=D), src[:, off : off + cw]
                    )
                    nc.scalar.activation(
                        nxt[:, off : off + cw],
                        pt[:D, :],
                        AF.Relu if l < 4 else AF.Identity,
                        bias=S("pb", crow=l, cn=1),
                    )
                cur, nxt = nxt, (xt_b if nxt is xt_a else xt_a)
            pre_t = cur  # [68, L] = pre(window rows)^T, var blocks of VBl cols

            # ---- UX = xs @ wx + b (+ warm-up mask), per pair, [128, L] ----
            ux = []
            for p in range(2):
                uxp = wp.tile([NCP, L], F32, tag=f"ux{p}")
                nc.vector.memset(uxp[:], 0.0)
                ux3 = uxp[:].rearrange("q (r j) -> q r j", j=4)
                for o in range(2):
                    vg = 2 * p + o
                    rows = slice(o * VOFF, o * VOFF + NCELLS)
                    for j in range(4):
                        s = vg * 4 + j
                        for off, cw in _chunks(VBl):
                            pt = psb.tile([128, cw], F32, tag="psB")
                            nc.tensor.matmul(
                                pt[:],
                                S("wxj", crow=s * NCP, cn=NCP),
                                pre_t[:, vg * VBl + off : vg * VBl + off + cw],
                            )
                            nc.scalar.activation(
                                ux3[rows, off : off + cw, j],
                                pt[rows, :],
                                AF.Identity,
                                bias=S("bv", prow=o * VOFF, pn=NCELLS, crow=p, cn=1),
                            )
                # warm-up mask: mfl = MASKVAL on core 0, 0 elsewhere
                nc.vector.tensor_scalar(
                    uxp[:, :W], uxp[:, :W], 1.0, S("mfl"), op0=OP.mult, op1=OP.add
                )
                ux.append(uxp)

            # ---- LTC scans: FOUR interleaved chains (pair p, half s) ----
            # Each core splits its SEG columns into two halves, each scanned by
            # an independent chain with its own W-step warm-up from h=0 (the
            # second half's warm-up re-reads columns the first half also
            # covers, so the ux window is unchanged). Four chains hide each
            # other's cross-engine latency; the scalar engine's 4 sigmoids per
            # step become the throughput limit instead of chain latency.
            HSEG = SEG // 2
            LP = W + HSEG
            hb = [
                [
                    wp.tile(
                        [NCP, LP + (PRED_N if s == 1 else 0)],
                        F32, tag=f"hbuf{p}_{s}", name=f"hbuf{p}_{s}",
                    )
                    for s in range(2)
                ]
                for p in range(2)
            ]
            h0 = cp.tile([NCP, 1], F32, tag="h0")
            nc.vector.memset(h0[:], 0.0)

            def cell(p, s, hprev, bias_ap, dst, extra_mm=None):
                # m-state form (mstate=True): the recurrence is rewritten in
                # m = h - a*C, which satisfies  m' = m*r + B  with
                # r = 1/(C + DT*f), B = -a*(C-1), and f = sigmoid(wh^T m + ux')
                # where ux' absorbs wh^T(a*C) and the output bias absorbs
                # (a*C)@wout (all folded host-side). This removes the separate
                # numerator op: 3 DVE ops per cell instead of 4 on the hot
                # engine. Exact same fp32 dynamics up to rounding.
                pz = psc[p].tile([NCP, 1], F32, tag=f"psS{p}_{s}")
                if extra_mm is not None:
                    nc.tensor.matmul(
                        pz[:], S("wx2", crow=p * NCP, cn=NCP), extra_mm,
                        start=True, stop=False,
                    )
                nc.tensor.matmul(
                    pz[:], S("wh", crow=p * NCP, cn=NCP), hprev,
                    start=(extra_mm is None), stop=True,
                )
                ft = sm[p].tile([NCP, 1], F32, tag=f"f{p}_{s}")
                nc.scalar.activation(ft[:], pz[:], AF.Sigmoid, bias=bias_ap)
                dent = sm[p].tile([NCP, 1], F32, tag=f"den{p}_{s}")
                if den_act and s == 1:
                    # rebalance: den = DT*f + C as an Identity activation
                    # (scale+bias) moves work from the hot DVE onto ACT
                    nc.scalar.activation(
                        dent[:], ft[:], AF.Identity,
                        bias=C_sb[:, p : p + 1], scale=DT,
                    )
                else:
                    nc.vector.tensor_scalar(
                        dent[:], ft[:], DT, C_sb[:, p : p + 1],
                        op0=OP.mult, op1=OP.add,
                    )
                nc.vector.reciprocal(dent[:], dent[:])
                if mstate:
                    nc.vector.scalar_tensor_tensor(
                        dst, hprev, dent[:], S("Bv", crow=p, cn=1),
                        op0=OP.mult, op1=OP.add,
                    )
                else:
                    numt = sm[p].tile([NCP, 1], F32, tag=f"num{p}_{s}")
                    nc.vector.scalar_tensor_tensor(
                        numt[:], ft[:], A_sb[:, p : p + 1], hprev,
                        op0=OP.mult, op1=OP.add,
                    )
                    eng = nc.gpsimd if pool_mult else nc.vector
                    eng.tensor_tensor(dst, numt[:], dent[:], op=OP.mult)

            if skip_scan:
                for p in range(2):
                    for s in range(2):
                        nc.vector.memset(hb[p][s][:], 0.0)
            for rep in range(0 if skip_scan else scan_repeat):
                for t in range(LP):
                    for s in range(2):
                        for p in range(2):
                            if t == 0:
                                init = S("m0", crow=p, cn=1) if mstate else h0[:]
                                hprev = init if rep == 0 else hb[p][s][:, LP - 1 : LP]
                            else:
                                hprev = hb[p][s][:, t - 1 : t]
                            cell(
                                p, s, hprev,
                                ux[p][:, s * HSEG + t : s * HSEG + t + 1],
                                hb[p][s][:, t : t + 1],
                            )

            # ---- batched output projection of the segment columns ----
            vvt = [
                wp.tile([VOFF, SEGE], F32, tag=f"vvt{p}", name=f"vvt{p}")
                for p in range(2)
            ]
            for p in range(2):
                for s in range(2):
                    for off, cw in _chunks(HSEG):
                        pv = psb.tile([128, cw], F32, tag="psB")
                        nc.tensor.matmul(
                            pv[:VOFF, :],
                            S("wout", crow=p * VOFF, cn=VOFF),
                            hb[p][s][:, W + off : W + off + cw],
                        )
                        nc.scalar.activation(
                            vvt[p][:, s * HSEG + off : s * HSEG + off + cw],
                            pv[:VOFF, :],
                            AF.Identity, bias=S("bo", crow=p, cn=1),
                        )

            # ---- autoregressive prediction (only core 7's result is used) ----
            for i in range(PRED_N):
                for p in range(2):
                    tl = LP + i
                    vprev = vvt[p][:, SEG + i - 1 : SEG + i]
                    cell(
                        p, 1,
                        hb[p][1][:, tl - 1 : tl],
                        S("bv", crow=p, cn=1),
                        hb[p][1][:, tl : tl + 1],
                        extra_mm=vprev,
                    )
                    pv = psc[p].tile([NCP, 1], F32, tag=f"psS{p}_1")
                    nc.tensor.matmul(
                        pv[:VOFF, :], S("wout", crow=p * VOFF, cn=VOFF),
                        hb[p][1][:, tl : tl + 1],
                    )
                    nc.scalar.activation(
                        vvt[p][:, SEG + i : SEG + i + 1], pv[:VOFF, :],
                        AF.Identity, bias=S("bo", crow=p, cn=1),
                    )

            # ---- gather per-var views (var-odd needs a lane move via DMA) ----
            vsh = []
            for p in range(2):
                vsh.append(vvt[p][0:LEVELS, :])
                tv = wp.tile([LEVELS, SEGE], F32, tag=f"vshB{p}")
                nc.sync.dma_start(tv[:], vvt[p][32 : 32 + LEVELS, :])
                vsh.append(tv[:])

            # ---- c1 encoder per var (all 5 layers relu'd: 1-4 inner, 5 outer) ----
            y5 = []
            for v in range(VAR_N):
                src = vsh[v]
                for l in range(1, 6):
                    fo = C1_DIMS[l - 1][1]
                    dst = wp.tile([fo, SEGE], F32, tag=f"c1y{l}_{v}")
                    for off, cw in _chunks(SEGE):
                        pt = psb.tile([128, cw], F32, tag="psB")
                        nc.tensor.matmul(
                            pt[:fo, :], S(f"c1w{l}"), src[:, off : off + cw]
                        )
                        nc.scalar.activation(
                            dst[:, off : off + cw], pt[:fo, :], AF.Relu,
                            bias=S("c1b", pn=fo, crow=l - 1, cn=1),
                        )
                    src = dst[:]
                y5.append(src)  # [68, SEGE]

            # ---- c2 encoder ----
            acts = y5
            for l in range(1, 6):
                fi, fo = C2_DIMS[l - 1]
                msplit = MSPLIT_272 if fo == FLAT else [(0, fo)]
                newacts = []
                for mi, (mo, mw) in enumerate(msplit):
                    dst = wp.tile([mw, SEGE], F32, tag=f"c2z{l}_{mi}")
                    for off, cw in _chunks(SEGE):
                        pt = psb.tile([128, cw], F32, tag="psB")
                        n_k = len(acts)
                        for ki, atile in enumerate(acts):
                            nc.tensor.matmul(
                                pt[:mw, :],
                                S(f"c2w{l}_{ki}", crow=mo, cn=mw),
                                atile[:, off : off + cw],
                                start=(ki == 0),
                                stop=(ki == n_k - 1),
                            )
                        bias = (
                            S(f"c2bs{l}", pn=mw, crow=mi, cn=1)
                            if fo == FLAT
                            else S(f"c2b{l}")
                        )
                        nc.scalar.activation(
                            dst[:, off : off + cw],
                            pt[:mw, :],
                            AF.Relu if l < 5 else AF.Identity,
                            bias=bias,
                        )
                    newacts.append(dst[:])
                acts = newacts

            nc.sync.dma_start(d_out.ap(), acts[0])

    nc.compile()
    return nc, dict(
        T=T, TP=T + PRED_N, SEG=SEG, W=W, L=L, VBl=VBl, SEGE=SEGE,
        n_cores=n_cores, pk=pk, NCOL=NCOL, mstate=mstate,
    )


def make_in_maps(inputs, meta):
    """Host-side layout: pack everything into one [128, NCOL] array per core."""
    T, SEG, W, L, VBl = meta["T"], meta["SEG"], meta["W"], meta["L"], meta["VBl"]
    n_cores, pk, NCOL = meta["n_cores"], meta["pk"], meta["NCOL"]
    g = lambda k: np.ascontiguousarray(np.asarray(inputs[k], dtype=np.float32))

    base = np.zeros((128, NCOL), np.float32)

    def put(name, arr):
        parts, c0, ncol = pk.slots[name]
        assert arr.shape == (parts, ncol), (name, arr.shape, (parts, ncol))
        base[:parts, c0 : c0 + ncol] = arr

    x = g("x")
    put("pw", np.stack([g(f"pw{i}") for i in range(1, 6)]).transpose(1, 0, 2).reshape(D, 5 * D))
    put("pb", np.stack([g(f"pb{i}") for i in range(1, 6)]).T)
    wx_all = g("ltc_wx")  # (4, 17, 56)

    wxj = np.zeros((16, D, NCP), np.float32)
    for vg in range(4):
        for j in range(4):
            wxj[vg * 4 + j, 17 * j : 17 * (j + 1), (vg % 2) * VOFF : (vg % 2) * VOFF + NCELLS] = wx_all[vg]
    put("wxj", wxj.transpose(1, 0, 2).reshape(D, 16 * NCP))

    mstate = meta.get("mstate", True)
    DTc = np.float32(DT)
    WH = np.zeros((NCP, 2, NCP), np.float32)
    WX2P = np.zeros((VOFF, 2, NCP), np.float32)
    WOUTP = np.zeros((NCP, 2, VOFF), np.float32)
    av = np.zeros((NCP, 2), np.float32)
    tau = np.full((NCP, 2), 0.5, np.float32)
    bv = np.zeros((NCP, 2), np.float32)
    bo = np.zeros((VOFF, 2), np.float32)
    Bv = np.zeros((NCP, 2), np.float32)
    m0 = np.zeros((NCP, 2), np.float32)
    for p in range(2):
        for o in range(2):
            v = 2 * p + o
            sl = slice(o * VOFF, o * VOFF + NCELLS)
            WH[sl, p, sl] = g("ltc_wh")[v]
            WX2P[o * 32 : o * 32 + LEVELS, p, sl] = wx_all[v]
            WOUTP[sl, p, o * 32 : o * 32 + LEVELS] = g("ltc_wout")[v]
            av[sl, p] = g("ltc_a")[v]
            tau[sl, p] = g("ltc_tau")[v]
            bvec = g("ltc_b")[v]
            bout = g("ltc_bout")[v]
            if mstate:
                # fold the affine state shift h = m + a*C into the biases
                a_v = g("ltc_a")[v]
                C_v = (1.0 + DTc / (g("ltc_tau")[v] + 0.5)).astype(np.float32)
                aC = (a_v * C_v).astype(np.float32)
                Bv[sl, p] = -a_v * (C_v - 1.0)
                m0[sl, p] = -aC
                bvec = bvec + aC @ g("ltc_wh")[v]
                bout = bout + aC @ g("ltc_wout")[v]
            bv[sl, p] = bvec
            bo[o * 32 : o * 32 + LEVELS, p] = bout
    put("wh", WH.reshape(NCP, 2 * NCP))
    put("wx2", WX2P.reshape(VOFF, 2 * NCP))
    put("wout", WOUTP.reshape(NCP, 2 * VOFF))
    put("av", av)
    put("tau", tau)
    put("bv", bv)
    put("bo", bo)
    put("Bv", Bv)
    put("m0", m0)

    for i in range(1, 6):
        put(f"c1w{i}", g(f"c1w{i}"))
    c1b = np.zeros((D, 5), np.float32)
    for i in range(1, 6):
        b = g(f"c1b{i}")
        c1b[: b.shape[0], i - 1] = b
    put("c1b", c1b)

    for i in range(1, 6):
        w = g(f"c2w{i}")
        for ki, (ko, kw) in enumerate(C2_KSPLITS[i]):
            put(f"c2w{i}_{ki}", w[ko : ko + kw])
    for i in (1, 2):
        b = g(f"c2b{i}")
        bs = np.zeros((128, 3), np.float32)
        for mi, (mo, mw) in enumerate(MSPLIT_272):
            bs[:mw, mi] = b[mo : mo + mw]
        put(f"c2bs{i}", bs)
    for i in (3, 4, 5):
        put(f"c2b{i}", g(f"c2b{i}")[:, None])

    # x reshaped per var: pre row r of var v lives at x row v*(T//4)+r
    TB = T // 4
    maps = []
    for c in range(n_cores):
        m = base.copy()
        t0 = SEG * (c + 1) - L  # window start (may be negative)
        r0 = t0 // 4
        xw = np.zeros((4, VBl, D), np.float32)
        lo = max(0, -r0)
        xw[:, lo:] = x.reshape(4, TB, D)[:, r0 + lo : r0 + VBl]
        parts, c0, ncol = pk.slots["xw"]
        m[:parts, c0 : c0 + ncol] = xw.reshape(4 * VBl, D).T
        parts, c0, ncol = pk.slots["mfl"]
        m[:parts, c0] = MASKVAL if t0 < 0 else 0.0
        maps.append({"cst": m})
    return maps


_CACHE = {}


def _get_built(T=T_FULL):
    if T not in _CACHE:
        _CACHE[T] = build(T)
    return _CACHE[T]


def kernel(**inputs) -> np.ndarray:
    nc, meta = _get_built(T_FULL)
    in_maps = make_in_maps(inputs, meta)
    res = bass_utils.run_bass_kernel_spmd(
        nc, in_maps, core_ids=list(range(meta["n_cores"]))
    )
    SEG = meta["SEG"]
    parts = [res.results[c]["out"][:, :SEG] for c in range(meta["n_cores"] - 1)]
    parts.append(res.results[meta["n_cores"] - 1]["out"])  # includes the 12 pred cols
    full = np.concatenate(parts, axis=1).T  # (T+12, 68)
    return np.ascontiguousarray(full)
